# revision 10
# baseline (speedup 1.0000x reference)
"""Trainium2 Bass kernel for nn_DiversityLoss.

loss = mean_{i<j} exp(-0.1 * ||x_i - x_j||)  for x = outputs [8192, 64] fp32.

Strategy (8 NeuronCores, SPMD — one NEFF, per-core data):
  * Augmented-matmul trick: with u_i = [x_i, |x_i|^2, 1] and
    v_j = [-2 x_j, 1, |x_j|^2] (K = 66), a PE matmul tile produces
    squared pairwise distances directly in PSUM.
  * Row sharding: 16 row-blocks of 512; core m owns blocks {m, 15-m}.
    Block r covers column blocks r..15 (block-level upper triangle), so
    every core processes exactly 17 column-tiles of 512 — a uniform
    instruction stream; only the DMA'd data differs per core.
  * Mixed precision: the two diagonal tiles (t=0,1) run exact fp32
    matmuls — the diagonal s_ii ~ 0 requires full-precision products to
    stay within the sqrt bias — and double as PE warm-up. The 15
    off-diagonal tiles run bf16 matmuls (full PE rate, half the DMA
    bytes); their per-element noise (~±0.9 on s ~ 30..300) averages out
    across 33M pairs (validated ~2e-6 end-to-end).
  * ACT pass 1 per tile: d = sqrt(s + 1e-3) staged to SBUF as bf16 (the
    bias keeps the diagonal away from sqrt(<0) — no mask, no relu).
    ACT pass 2: exp(-0.1 d) with hardware accumulation. sqrt/exp live in
    different ACT table sets, so batching all sqrts then all exps pays
    only 2 table loads.
  * Diagonal 512-blocks are computed in full; the host subtracts the
    analytic diagonal N*exp(-0.1*sqrt(BIAS)) and halves (symmetry).
  * Raw Bass (no Tile framework): this container's walrus accepts only
    one sync-wait per instruction, so every wait is an explicit wait_ge.
"""

import os
import sys

import numpy as np

_TRN_REPO = "/opt/trn_rl_repo"
if _TRN_REPO not in sys.path:
    sys.path.insert(0, _TRN_REPO)

N = 8192
D = 64
K = D + 2  # 66
NB = 16  # number of 512-row blocks
BS = 512  # block size
NCORES = 8
TILES = 17  # column tiles per core (uniform across cores)
NDIAG = 2  # diagonal tiles per core
PF = 2048  # psum tile free dim = 4 matmuls of 512
DCOLS = TILES * PF  # 34816 staged-d columns
BIAS = 1e-3
DIAG_BIAS = 1e-3
SCALE = 0.1

_CACHE = {}


def _to_bf16(a: np.ndarray) -> np.ndarray:
    """fp32 -> bf16 (RNE) as an ml_dtypes.bfloat16 array."""
    import ml_dtypes

    return np.ascontiguousarray(a, dtype=np.float32).astype(ml_dtypes.bfloat16)


def _build_bass():
    import concourse.bass as bass
    import concourse.mybir as mybir

    f32 = mybir.dt.float32
    bf16 = mybir.dt.bfloat16
    AF = mybir.ActivationFunctionType

    nc = bass.Bass()
    wv32_d = nc.declare_dram_parameter(
        "wv32", [K, NDIAG * 2 * BS], f32, isOutput=False
    )
    wv16_d = nc.declare_dram_parameter(
        "wv16", [K, (TILES - NDIAG) * 2 * BS], bf16, isOutput=False
    )
    b_d = nc.declare_dram_parameter("b", [128, 2], f32, isOutput=False)
    out_d = nc.declare_dram_parameter("out", [128, 2], f32, isOutput=True)

    with (
        nc.sbuf_tensor([K, NDIAG * 2 * BS], f32) as wv32_sb,
        nc.sbuf_tensor([K, (TILES - NDIAG) * 2 * BS], bf16) as wv16_sb,
        nc.sbuf_tensor([128, DCOLS], bf16) as d_sb,
        nc.sbuf_tensor([128, 2], f32) as b_sb,
        nc.sbuf_tensor([128, 2], f32) as acc_sb,
        nc.sbuf_tensor([128, 2], f32) as fence_sb,
        nc.psum_tensor([128, PF], f32) as ps0,
        nc.psum_tensor([128, PF], f32) as ps1,
        nc.semaphore("dma_sem") as dma_sem,
        nc.semaphore("pe_sem") as pe_sem,
        nc.semaphore("act_sem") as act_sem,
        nc.Block() as block,
    ):
        ps = [ps0, ps1]

        @block.sync
        def _(sync):
            sync.dma_start(out=b_sb[:], in_=b_d[:]).then_inc(dma_sem, 16)
            sync.dma_start(out=wv32_sb[:], in_=wv32_d[:]).then_inc(dma_sem, 16)
            for j in range(TILES - NDIAG):
                sl = slice(j * 2 * BS, (j + 1) * 2 * BS)
                sync.dma_start(out=wv16_sb[:, sl], in_=wv16_d[:, sl]).then_inc(
                    dma_sem, 16
                )
            # Trailing fence DMAs: a transfer's completion semaphore can fire
            # before all of its split descriptor streams have landed, so
            # consumers wait 2 DMAs past the one carrying their data.
            sync.dma_start(out=fence_sb[:], in_=b_d[:]).then_inc(dma_sem, 16)
            sync.dma_start(out=fence_sb[:], in_=b_d[:]).then_inc(dma_sem, 16)
            sync.wait_ge(act_sem, TILES + 1)
            sync.dma_start(out=out_d[:], in_=acc_sb[:]).then_inc(dma_sem, 16)

        @block.tensor
        def _(tensor):
            for t in range(TILES):
                # data availability: b + wv32 at 32; wv16 tile j=t-2 at
                # 16*(t+1); +32 slack for straggling descriptor streams
                tensor.wait_ge(dma_sem, (32 if t < NDIAG else 16 * (t + 1)) + 32)
                if t >= 2:
                    # psum buffer t%2 reusable once sqrt(t-2) retired
                    tensor.wait_ge(act_sem, t - 1)
                p = ps[t % 2]
                if t < NDIAG:
                    base = t * 2 * BS
                    vt = wv32_sb[:, base + BS : base + 2 * BS]
                    wsrc = wv32_sb
                else:
                    base = (t - NDIAG) * 2 * BS
                    vt = wv16_sb[:, base + BS : base + 2 * BS]
                    wsrc = wv16_sb
                mm = None
                for q in range(4):
                    wt = wsrc[:, base + q * 128 : base + (q + 1) * 128]
                    mm = nc.tensor.matmul(p[:, q * BS : (q + 1) * BS], wt, vt)
                mm.then_inc(pe_sem, 1)

        @block.scalar
        def _(scalar):
            for t in range(TILES):
                scalar.wait_ge(pe_sem, t + 1)
                b = b_sb[:, 0:1] if t < NDIAG else b_sb[:, 1:2]
                nc.scalar.activation(
                    d_sb[:, t * PF : (t + 1) * PF],
                    ps[t % 2][:, :],
                    AF.Sqrt,
                    bias=b,
                ).then_inc(act_sem, 1)
            nc.scalar.activation(
                d_sb[:, : NDIAG * PF],
                d_sb[:, : NDIAG * PF],
                AF.Exp,
                scale=-SCALE,
                accum_out=acc_sb[:, 0:1],
            )
            nc.scalar.activation(
                d_sb[:, NDIAG * PF :],
                d_sb[:, NDIAG * PF :],
                AF.Exp,
                scale=-SCALE,
                accum_out=acc_sb[:, 1:2],
            ).then_inc(act_sem, 1)

    return nc


def _pack_inputs(X: np.ndarray):
    """Per-core packed [w-tile || v-tile] operand buffers (fp32 diag tiles,
    bf16 off-diag tiles)."""
    X = np.ascontiguousarray(X, dtype=np.float32)
    sq = (X.astype(np.float64) ** 2).sum(axis=1)
    sq32 = sq.astype(np.float32)
    ones = np.ones((N, 1), np.float32)
    U = np.concatenate([X, sq32[:, None], ones], axis=1)  # [N, 66]
    V = np.concatenate([-2.0 * X, ones, sq32[:, None]], axis=1)  # [N, 66]
    UT = np.ascontiguousarray(U.T)  # [66, N] fp32
    VT = np.ascontiguousarray(V.T)
    UT16 = _to_bf16(UT)  # [66, N] bf16
    VT16 = _to_bf16(VT)

    b = np.empty((128, 2), np.float32)
    b[:, 0] = DIAG_BIAS
    b[:, 1] = BIAS

    in_maps = []
    for m in range(NCORES):
        a, c = m, NB - 1 - m
        # diagonal tiles first, then off-diagonal (row_block, col_block)
        off = [(a, j) for j in range(a + 1, NB)] + [(c, j) for j in range(c + 1, NB)]
        assert len(off) == TILES - NDIAG
        wv32 = np.empty((K, NDIAG * 2 * BS), np.float32)
        for t, rb in enumerate((a, c)):
            base = t * 2 * BS
            wv32[:, base : base + BS] = UT[:, rb * BS : (rb + 1) * BS]
            wv32[:, base + BS : base + 2 * BS] = VT[:, rb * BS : (rb + 1) * BS]
        import ml_dtypes
        wv16 = np.empty((K, (TILES - NDIAG) * 2 * BS), ml_dtypes.bfloat16)
        for j, (rb, cb) in enumerate(off):
            base = j * 2 * BS
            wv16[:, base : base + BS] = UT16[:, rb * BS : (rb + 1) * BS]
            wv16[:, base + BS : base + 2 * BS] = VT16[:, cb * BS : (cb + 1) * BS]
        in_maps.append({"wv32": wv32, "wv16": wv16, "b": b})
    return in_maps


def _combine(outs):
    """Host-side unshard: reduce per-core [128, 2] partials to the loss."""
    total_diag = 0.0
    total_off = 0.0
    for o in outs:
        o = np.asarray(o, dtype=np.float64)
        total_diag += o[:, 0].sum()
        total_off += o[:, 1].sum()
    diag_terms = N * float(np.exp(-SCALE * np.sqrt(DIAG_BIAS)))
    s = total_off + (total_diag - diag_terms) / 2.0
    n_pairs = N * (N - 1) / 2.0
    return np.float32(s / n_pairs)


def kernel(outputs: np.ndarray) -> np.ndarray:
    from concourse.bass_utils import run_bass_kernel_spmd

    if "nc" not in _CACHE:
        _CACHE["nc"] = _build_bass()
    nc = _CACHE["nc"]

    in_maps = _pack_inputs(np.asarray(outputs))
    res = run_bass_kernel_spmd(nc, in_maps, list(range(NCORES)))
    outs = [res.results[i]["out"] for i in range(NCORES)]
    return _combine(outs)


if __name__ == "__main__":
    x = np.random.randn(N, D).astype(np.float32)
    print(kernel(x))


# revision 14
# speedup vs baseline: 981.1223x; 981.1223x over previous
"""Trainium2 Bass kernel for nn_DiversityLoss.

loss = mean_{i<j} exp(-0.1 * ||x_i - x_j||)  for x = outputs [8192, 64] fp32.

Strategy (8 NeuronCores, SPMD — one NEFF, per-core data):
  * Augmented-matmul trick: with u_i = [x_i, |x_i|^2, 1] and
    v_j = [-2 x_j, 1, |x_j|^2] (K = 66), a PE matmul tile produces
    squared pairwise distances directly in PSUM.
  * Row sharding: 16 row-blocks of 512; core m owns blocks {m, 15-m}.
    Block r covers column blocks r..15 (block-level upper triangle), so
    every core processes exactly 17 column-tiles of 512 — a uniform
    instruction stream; only the DMA'd data differs per core.
  * Mixed precision: the two diagonal tiles (scheduled last, t=15,16)
    run exact fp32 matmuls — the diagonal s_ii ~ 0 requires
    full-precision products to stay within the sqrt bias. The 15
    off-diagonal tiles (t=0..14) run bf16 matmuls (full PE rate, half
    the DMA bytes); their per-element noise (~±0.9 on s ~ 30..300)
    averages out across 33M pairs (validated ~2e-6 end-to-end).
  * ACT pass 1 per tile: d = sqrt(s + 1e-3) staged to SBUF as bf16 (the
    bias keeps the diagonal away from sqrt(<0) — no mask, no relu).
    ACT pass 2: exp(-0.1 d) with hardware accumulation. sqrt/exp live in
    different ACT table sets, so batching all sqrts then all exps pays
    only 2 table loads.
  * Diagonal 512-blocks are computed in full; the host subtracts the
    analytic diagonal N*exp(-0.1*sqrt(BIAS)) and halves (symmetry).
  * Raw Bass (no Tile framework): this container's walrus accepts only
    one sync-wait per instruction, so every wait is an explicit wait_ge.
"""

import os
import sys

import numpy as np

_TRN_REPO = "/opt/trn_rl_repo"
if _TRN_REPO not in sys.path:
    sys.path.insert(0, _TRN_REPO)

N = 8192
D = 64
K = D + 2  # 66
NB = 16  # number of 512-row blocks
BS = 512  # block size
NCORES = 8
TILES = 17  # column tiles per core (uniform across cores)
NDIAG = 2  # diagonal tiles per core
PF = 2048  # psum tile free dim = 4 matmuls of 512
DCOLS = TILES * PF  # 34816 staged-d columns
BIAS = 1e-3
DIAG_BIAS = 1e-3
SCALE = 0.1

_CACHE = {}


def _to_bf16(a: np.ndarray) -> np.ndarray:
    """fp32 -> bf16 (RNE) as an ml_dtypes.bfloat16 array."""
    import ml_dtypes

    return np.ascontiguousarray(a, dtype=np.float32).astype(ml_dtypes.bfloat16)


def _build_bass():
    import concourse.bass as bass
    import concourse.mybir as mybir

    f32 = mybir.dt.float32
    bf16 = mybir.dt.bfloat16
    AF = mybir.ActivationFunctionType

    nc = bass.Bass()
    wv32_d = nc.declare_dram_parameter(
        "wv32", [K, NDIAG * 2 * BS], f32, isOutput=False
    )
    wv16_d = nc.declare_dram_parameter(
        "wv16", [K, (TILES - NDIAG) * 2 * BS], bf16, isOutput=False
    )
    b_d = nc.declare_dram_parameter("b", [128, 2], f32, isOutput=False)
    out_d = nc.declare_dram_parameter("out", [128, 2], f32, isOutput=True)

    with (
        nc.sbuf_tensor([K, NDIAG * 2 * BS], f32) as wv32_sb,
        nc.sbuf_tensor([K, (TILES - NDIAG) * 2 * BS], bf16) as wv16_sb,
        nc.sbuf_tensor([128, DCOLS], bf16) as d_sb,
        nc.sbuf_tensor([128, 2], f32) as b_sb,
        nc.sbuf_tensor([128, 2], f32) as acc_sb,
        nc.sbuf_tensor([128, 2], f32) as fence_sb,
        nc.psum_tensor([128, PF], f32) as ps0,
        nc.psum_tensor([128, PF], f32) as ps1,
        nc.semaphore("dma_sem") as dma_sem,
        nc.semaphore("pe_sem") as pe_sem,
        nc.semaphore("act_sem") as act_sem,
        nc.Block() as block,
    ):
        ps = [ps0, ps1]

        @block.sync
        def _(sync):
            # A transfer's completion semaphore can fire before all of its
            # split descriptor streams have landed, so consumers wait 2 DMAs
            # past the one carrying their data. The two tiny fences after
            # tile 0 make that slack cheap for the pipeline start; two more
            # at the end cover the last data transfers.
            sync.dma_start(out=b_sb[:], in_=b_d[:]).then_inc(dma_sem, 16)
            sync.dma_start(
                out=wv16_sb[:, 0 : 2 * BS], in_=wv16_d[:, 0 : 2 * BS]
            ).then_inc(dma_sem, 16)
            sync.dma_start(out=fence_sb[:], in_=b_d[:]).then_inc(dma_sem, 16)
            sync.dma_start(out=fence_sb[:], in_=b_d[:]).then_inc(dma_sem, 16)
            for j in range(1, TILES - NDIAG):
                sl = slice(j * 2 * BS, (j + 1) * 2 * BS)
                sync.dma_start(out=wv16_sb[:, sl], in_=wv16_d[:, sl]).then_inc(
                    dma_sem, 16
                )
            sync.dma_start(out=wv32_sb[:], in_=wv32_d[:]).then_inc(dma_sem, 16)
            sync.dma_start(out=fence_sb[:], in_=b_d[:]).then_inc(dma_sem, 16)
            sync.dma_start(out=fence_sb[:], in_=b_d[:]).then_inc(dma_sem, 16)
            sync.wait_ge(act_sem, TILES + 2)
            sync.dma_start(out=out_d[:], in_=acc_sb[:]).then_inc(dma_sem, 16)

        @block.tensor
        def _(tensor):
            noff = TILES - NDIAG
            for t in range(TILES):
                # data availability (+32 slack for straggling descriptor
                # streams): wv16 tile t at 16*(t+2); wv32 at 16*(noff+3)
                if t == 0:
                    tensor.wait_ge(dma_sem, 64)
                else:
                    tensor.wait_ge(
                        dma_sem,
                        (16 * (t + 4) if t < noff else 16 * (noff + 4)) + 32,
                    )
                if t >= 2:
                    # psum buffer t%2 reusable once sqrt(t-2) retired
                    tensor.wait_ge(act_sem, t)
                p = ps[t % 2]
                if t >= noff:
                    base = (t - noff) * 2 * BS
                    vt = wv32_sb[:, base + BS : base + 2 * BS]
                    wsrc = wv32_sb
                else:
                    base = t * 2 * BS
                    vt = wv16_sb[:, base + BS : base + 2 * BS]
                    wsrc = wv16_sb
                mm = None
                for q in range(4):
                    wt = wsrc[:, base + q * 128 : base + (q + 1) * 128]
                    mm = nc.tensor.matmul(p[:, q * BS : (q + 1) * BS], wt, vt)
                mm.then_inc(pe_sem, 1)

        @block.scalar
        def _(scalar):
            # dummy: pulls the ~2.7us sqrt table load into the DMA lead-in
            nc.scalar.activation(
                fence_sb[:, 0:1], fence_sb[:, 0:1], AF.Sqrt
            ).then_inc(act_sem, 1)
            noff = TILES - NDIAG
            for t in range(TILES):
                scalar.wait_ge(pe_sem, t + 1)
                nc.scalar.activation(
                    d_sb[:, t * PF : (t + 1) * PF],
                    ps[t % 2][:, :],
                    AF.Sqrt,
                    bias=b_sb[:, 0:1],
                ).then_inc(act_sem, 1)
            nc.scalar.activation(
                d_sb[:, : noff * PF],
                d_sb[:, : noff * PF],
                AF.Exp,
                scale=-SCALE,
                accum_out=acc_sb[:, 1:2],
            )
            nc.scalar.activation(
                d_sb[:, noff * PF :],
                d_sb[:, noff * PF :],
                AF.Exp,
                scale=-SCALE,
                accum_out=acc_sb[:, 0:1],
            ).then_inc(act_sem, 1)

    return nc


def _pack_inputs(X: np.ndarray):
    """Per-core packed [w-tile || v-tile] operand buffers (fp32 diag tiles,
    bf16 off-diag tiles)."""
    X = np.ascontiguousarray(X, dtype=np.float32)
    sq = (X.astype(np.float64) ** 2).sum(axis=1)
    sq32 = sq.astype(np.float32)
    ones = np.ones((N, 1), np.float32)
    U = np.concatenate([X, sq32[:, None], ones], axis=1)  # [N, 66]
    V = np.concatenate([-2.0 * X, ones, sq32[:, None]], axis=1)  # [N, 66]
    UT = np.ascontiguousarray(U.T)  # [66, N] fp32
    VT = np.ascontiguousarray(V.T)
    UT16 = _to_bf16(UT)  # [66, N] bf16
    VT16 = _to_bf16(VT)

    b = np.empty((128, 2), np.float32)
    b[:, 0] = DIAG_BIAS
    b[:, 1] = BIAS

    in_maps = []
    for m in range(NCORES):
        a, c = m, NB - 1 - m
        # diagonal tiles first, then off-diagonal (row_block, col_block)
        off = [(a, j) for j in range(a + 1, NB)] + [(c, j) for j in range(c + 1, NB)]
        assert len(off) == TILES - NDIAG
        wv32 = np.empty((K, NDIAG * 2 * BS), np.float32)
        for t, rb in enumerate((a, c)):
            base = t * 2 * BS
            wv32[:, base : base + BS] = UT[:, rb * BS : (rb + 1) * BS]
            wv32[:, base + BS : base + 2 * BS] = VT[:, rb * BS : (rb + 1) * BS]
        import ml_dtypes
        wv16 = np.empty((K, (TILES - NDIAG) * 2 * BS), ml_dtypes.bfloat16)
        for j, (rb, cb) in enumerate(off):
            base = j * 2 * BS
            wv16[:, base : base + BS] = UT16[:, rb * BS : (rb + 1) * BS]
            wv16[:, base + BS : base + 2 * BS] = VT16[:, cb * BS : (cb + 1) * BS]
        in_maps.append({"wv32": wv32, "wv16": wv16, "b": b})
    return in_maps


def _combine(outs):
    """Host-side unshard: reduce per-core [128, 2] partials to the loss."""
    total_diag = 0.0
    total_off = 0.0
    for o in outs:
        o = np.asarray(o, dtype=np.float64)
        total_diag += o[:, 0].sum()
        total_off += o[:, 1].sum()
    diag_terms = N * float(np.exp(-SCALE * np.sqrt(DIAG_BIAS)))
    s = total_off + (total_diag - diag_terms) / 2.0
    n_pairs = N * (N - 1) / 2.0
    return np.float32(s / n_pairs)


def kernel(outputs: np.ndarray) -> np.ndarray:
    from concourse.bass_utils import run_bass_kernel_spmd

    if "nc" not in _CACHE:
        _CACHE["nc"] = _build_bass()
    nc = _CACHE["nc"]

    in_maps = _pack_inputs(np.asarray(outputs))
    res = run_bass_kernel_spmd(nc, in_maps, list(range(NCORES)))
    outs = [res.results[i]["out"] for i in range(NCORES)]
    return _combine(outs)


if __name__ == "__main__":
    x = np.random.randn(N, D).astype(np.float32)
    print(kernel(x))
